# revision 10
# baseline (speedup 1.0000x reference)
"""Trainium2 Bass kernel for nn_AGITransformer140B (8-core tensor-parallel).

Transformer block: h = x + Attn(RMSNorm(x)); out = h + SwiGLU(RMSNorm(h)).

Key simplification: the reference's second attention pass uses
rotate_half(Q), rotate_half(K) — which preserves both Q·K and Q²·K² inner
products exactly, so out2 == out1 and the sigmoid gate is a no-op.  Only one
attention pass is computed.

Sharding: TP-8 over heads (2/core) and d_ff (1024/core).  Partial attention
and FFN outputs are ReduceScattered over tokens (4 chunks of 512 tokens,
each core owning 64-token slices); RMSNorm2 + residuals run on the local
token shard; normed activations are AllGathered (feature-major) for the FFN.

Layouts (per core):
  xT       [D=2048, T=2048]  bf16 feature-major input (host-transposed)
  xs       [256, 2048]       bf16 x token-shard, local (k,j) order
  wqT/wkT  [2048, 256]  bf16 (norm1_w folded, head-dim pi-permuted: evens|odds)
  wvT      [2048, 256]  bf16 (norm1_w folded)
  woT      [256, 2048]  bf16
  w1T/w3T  [8, 2048, 128] bf16 df-major (norm2_w folded)
  w2T      [1024, 2048] bf16
  ropeC*/S* [64, 2048]  bf16 rope tables (Q tables pre-scaled by 1/sqrt(hd))
Output: out [256, 2048] f32, core c owns tokens {k*512 + c*64 + j}.
"""

import os
import sys
import types

sys.path.insert(0, "/opt/trn_rl_repo")

# ---- NTFF profile hook (boot() skips it: antenv stub lacks axon_hooks) ----
if "antenv.axon_hooks" not in sys.modules:
    _hooks_mod = types.ModuleType("antenv.axon_hooks")
    _HOOK = [None]
    _hooks_mod.set_axon_ntff_profile_hook = lambda h: _HOOK.__setitem__(0, h)
    _hooks_mod.get_axon_ntff_profile_hook = lambda: _HOOK[0]
    sys.modules["antenv.axon_hooks"] = _hooks_mod
    try:
        from trn_agent_boot.trn_boot import _ntff_profile_via_ctypes

        _hooks_mod.set_axon_ntff_profile_hook(
            _ntff_profile_via_ctypes("/opt/axon/libaxon_pjrt.so")
        )
    except Exception:
        pass

import ml_dtypes
import numpy as np

import concourse.bass as bass
import concourse.mybir as mybir
import concourse.tile as tile
from concourse import bacc
from concourse.bass_utils import run_bass_kernel_spmd
from concourse.masks import make_identity

BF16 = ml_dtypes.bfloat16
F32 = mybir.dt.float32
BF = mybir.dt.bfloat16
AF = mybir.ActivationFunctionType
ALU = mybir.AluOpType

N_CORES = 8
B, S, D, NH, HD, DFF = 2, 1024, 2048, 16, 128, 8192
T = B * S                      # 2048 tokens
NHL = NH // N_CORES            # 2 heads per core
DQ = NHL * HD                  # 256
DFL = DFF // N_CORES           # 1024
NDF = DFL // 128               # 8 dff tiles per core
NK = 4                         # token chunks for RS pipelining
CHUNK = T // NK                # 512
SH = CHUNK // N_CORES          # 64 tokens per (chunk, core)
DT_TILES = D // 128            # 16
EPS = 1e-6
LAM = 0.1
HAD = 0.05
SQ_SCALE = float(np.sqrt(LAM * np.sqrt(HD)))   # fold lam*sqrt(hd) into Q^2
DEBUG = bool(int(os.environ.get("KERNEL_DEBUG", "0")))


def build_nc():
    nc = bacc.Bacc("TRN2", target_bir_lowering=False, debug=False)

    xT_e = nc.declare_dram_parameter("xT", [D, T], BF, isOutput=False)
    xs_e = nc.declare_dram_parameter("xs", [NK * SH, D], BF, isOutput=False)
    wqT_e = nc.declare_dram_parameter("wqT", [D, DQ], BF, isOutput=False)
    wkT_e = nc.declare_dram_parameter("wkT", [D, DQ], BF, isOutput=False)
    wvT_e = nc.declare_dram_parameter("wvT", [D, DQ], BF, isOutput=False)
    woT_e = nc.declare_dram_parameter("woT", [DQ, D], BF, isOutput=False)
    w1T_e = nc.declare_dram_parameter("w1T", [NDF, D, 128], BF, isOutput=False)
    w3T_e = nc.declare_dram_parameter("w3T", [NDF, D, 128], BF, isOutput=False)
    w2T_e = nc.declare_dram_parameter("w2T", [DFL, D], BF, isOutput=False)
    rCq_e = nc.declare_dram_parameter("ropeCq", [64, T], BF, isOutput=False)
    rSq_e = nc.declare_dram_parameter("ropeSq", [64, T], BF, isOutput=False)
    rCk_e = nc.declare_dram_parameter("ropeCk", [64, T], BF, isOutput=False)
    rSk_e = nc.declare_dram_parameter("ropeSk", [64, T], BF, isOutput=False)
    out_e = nc.declare_dram_parameter("out", [NK * SH, D], F32, isOutput=True)

    dbg = {}
    if DEBUG:
        dbg["xnt"] = nc.declare_dram_parameter("dbg_xnt", [D, T], BF, isOutput=True)
        dbg["qtr"] = nc.declare_dram_parameter("dbg_qtr", [DQ, T], BF, isOutput=True)
        dbg["ktr"] = nc.declare_dram_parameter("dbg_ktr", [DQ, T], BF, isOutput=True)
        dbg["v"] = nc.declare_dram_parameter("dbg_v", [T, DQ], BF, isOutput=True)
        dbg["ot"] = nc.declare_dram_parameter("dbg_ot", [DQ, T], BF, isOutput=True)
        dbg["attn"] = nc.declare_dram_parameter("dbg_attn", [T, D], BF, isOutput=True)
        dbg["h"] = nc.declare_dram_parameter("dbg_h", [NK * SH, D], F32, isOutput=True)
        dbg["y"] = nc.declare_dram_parameter("dbg_y", [T, D], BF, isOutput=True)

    RG = [list(range(N_CORES))]

    with tile.TileContext(nc) as tc:
        with tc.tile_pool(name="const", bufs=1) as const, \
             tc.tile_pool(name="dram", bufs=1, space="DRAM") as dram:
            ident = const.tile([128, 128], BF)
            make_identity(nc, ident)
            ones_c = const.tile([128, 1], BF)     # colsum lhsT (K=128, M=1)
            nc.vector.memset(ones_c[:], 1.0)
            ones_r = const.tile([1, 128], F32)    # bcast lhsT (K=1, M=128)
            nc.vector.memset(ones_r[:], 1.0)
            eps_t = const.tile([128, 1], F32)
            nc.vector.memset(eps_t[:], EPS)

            rCq = const.tile([64, T], BF)
            rSq = const.tile([64, T], BF)
            rCk = const.tile([64, T], BF)
            rSk = const.tile([64, T], BF)
            nc.sync.dma_start(rCq[:], rCq_e[:])
            nc.sync.dma_start(rSq[:], rSq_e[:])
            nc.sync.dma_start(rCk[:], rCk_e[:])
            nc.sync.dma_start(rSk[:], rSk_e[:])

            attn_rs = []
            with tc.tile_pool(name="aops", bufs=1) as aops:
                V = aops.tile([128, 16, DQ], BF, tag="v")
                QTr = aops.tile([128, NHL, T], BF, tag="qtr")
                KTr = aops.tile([128, NHL, T], BF, tag="ktr")
                Qsq = aops.tile([128, NHL, T], BF, tag="qsq")
                Ksq = aops.tile([128, NHL, T], BF, tag="ksq")
                OT = aops.tile([128, NHL, T], BF, tag="ot")

                with tc.tile_pool(name="xnt_pool", bufs=1) as xnt_pool:
                    XNT = xnt_pool.tile([128, DT_TILES, T], BF, tag="xnt")
                    nc.sync.dma_start(
                        XNT[:], xT_e.ap().rearrange("(dt p) t -> p dt t", p=128)
                    )
                    # ===== Phase 1: RMSNorm1 (feature-major, in-place) =====
                    with tc.tile_pool(name="n1", bufs=3) as n1, \
                         tc.tile_pool(name="n1s", bufs=1) as n1s, \
                         tc.tile_pool(name="ps_n1", bufs=1, space="PSUM") as ps_n1, \
                         tc.tile_pool(name="ps_rb1", bufs=2, space="PSUM") as ps_rb1:
                        ssq_ps = [
                            ps_n1.tile([1, 512], F32, tag=f"ssq{nn}", name=f"ssq{nn}")
                            for nn in range(4)
                        ]
                        for dt in range(DT_TILES):
                            sq = n1.tile([128, T], BF, tag="sq")
                            nc.scalar.activation(sq[:], XNT[:, dt, :], AF.Square)
                            for nn in range(4):
                                nc.tensor.matmul(
                                    ssq_ps[nn][:], ones_c[:, 0:1],
                                    sq[:, nn * 512:(nn + 1) * 512],
                                    start=(dt == 0), stop=(dt == DT_TILES - 1),
                                )
                        rstd = n1s.tile([1, T], F32, tag="rstd")
                        sr = n1s.tile([1, T], F32, tag="sr")
                        for nn in range(4):
                            nc.scalar.activation(
                                sr[:, nn * 512:(nn + 1) * 512], ssq_ps[nn][:],
                                AF.Sqrt, scale=1.0 / D, bias=eps_t[0:1, :],
                            )
                        nc.vector.reciprocal(rstd[:], sr[:])
                        RB1 = n1s.tile([128, T], BF, tag="rb1")
                        for nn in range(4):
                            rb_ps = ps_rb1.tile([128, 512], F32, tag="rbps")
                            nc.tensor.matmul(
                                rb_ps[:], ones_r[:, :],
                                rstd[:, nn * 512:(nn + 1) * 512],
                                start=True, stop=True,
                            )
                            nc.scalar.copy(RB1[:, nn * 512:(nn + 1) * 512], rb_ps[:])
                        for dt in range(DT_TILES):
                            nc.vector.tensor_mul(XNT[:, dt, :], XNT[:, dt, :], RB1[:])
                        if DEBUG:
                            nc.sync.dma_start(
                                dbg["xnt"].ap().rearrange("(dt p) t -> p dt t", p=128),
                                XNT[:],
                            )

                    # ===== Phase 2: QKV projections + RoPE =====
                    with tc.tile_pool(name="qkv_w", bufs=1) as qkv_w, \
                         tc.tile_pool(name="qkv_raw", bufs=1) as qkv_raw, \
                         tc.tile_pool(name="rope_tmp", bufs=1) as rtmp, \
                         tc.tile_pool(name="ps_qkv", bufs=2, space="PSUM") as ps_qkv:
                        WQ = qkv_w.tile([128, DT_TILES, DQ], BF, tag="wq")
                        WK = qkv_w.tile([128, DT_TILES, DQ], BF, tag="wk")
                        WV = qkv_w.tile([128, DT_TILES, DQ], BF, tag="wv")
                        nc.sync.dma_start(
                            WQ[:], wqT_e.ap().rearrange("(dt p) m -> p dt m", p=128))
                        nc.sync.dma_start(
                            WK[:], wkT_e.ap().rearrange("(dt p) m -> p dt m", p=128))
                        nc.sync.dma_start(
                            WV[:], wvT_e.ap().rearrange("(dt p) m -> p dt m", p=128))

                        QTraw = qkv_raw.tile([128, NHL, T], BF, tag="qraw")
                        KTraw = qkv_raw.tile([128, NHL, T], BF, tag="kraw")
                        for (W, OUT) in ((WQ, QTraw), (WK, KTraw)):
                            for hm in range(NHL):
                                for nq in range(4):
                                    ps = ps_qkv.tile([128, 512], F32, tag="qk_ps")
                                    for dt in range(DT_TILES):
                                        nc.tensor.matmul(
                                            ps[:],
                                            W[:, dt, hm * 128:(hm + 1) * 128],
                                            XNT[:, dt, nq * 512:(nq + 1) * 512],
                                            start=(dt == 0), stop=(dt == DT_TILES - 1),
                                        )
                                    nc.scalar.copy(
                                        OUT[:, hm, nq * 512:(nq + 1) * 512], ps[:])
                        for tt in range(16):
                            ps = ps_qkv.tile([128, DQ], F32, tag="v_ps")
                            for dt in range(DT_TILES):
                                nc.tensor.matmul(
                                    ps[:],
                                    XNT[:, dt, tt * 128:(tt + 1) * 128],
                                    WV[:, dt, :],
                                    start=(dt == 0), stop=(dt == DT_TILES - 1),
                                )
                            nc.scalar.copy(V[:, tt, :], ps[:])
                        if DEBUG:
                            nc.sync.dma_start(
                                dbg["v"].ap().rearrange("(tt p) m -> p tt m", p=128),
                                V[:],
                            )

                        # RoPE (pi layout: rows 0:64 = even dims, 64:128 = odds)
                        for (RAW, ROT, SQT, CC, SS, sc_sq) in (
                            (QTraw, QTr, Qsq, rCq, rSq, SQ_SCALE),
                            (KTraw, KTr, Ksq, rCk, rSk, 1.0),
                        ):
                            for h in range(NHL):
                                x1 = RAW[0:64, h, :]
                                x2c = rtmp.tile([64, T], BF, tag="x2c")
                                nc.vector.tensor_copy(x2c[:], RAW[64:128, h, :])
                                tA = rtmp.tile([64, T], BF, tag="ta")
                                tB = rtmp.tile([64, T], BF, tag="tb")
                                nc.vector.tensor_mul(tA[:], x1, CC[:])
                                nc.vector.tensor_mul(tB[:], x2c[:], SS[:])
                                nc.vector.tensor_sub(ROT[0:64, h, :], tA[:], tB[:])
                                tC = rtmp.tile([64, T], BF, tag="tc")
                                tD = rtmp.tile([64, T], BF, tag="td")
                                nc.vector.tensor_mul(tC[:], x1, SS[:])
                                nc.vector.tensor_mul(tD[:], x2c[:], CC[:])
                                hi = rtmp.tile([64, T], BF, tag="hi")
                                nc.vector.tensor_add(hi[:], tC[:], tD[:])
                                nc.vector.tensor_copy(ROT[64:128, h, :], hi[:])
                                nc.scalar.activation(
                                    SQT[:, h, :], ROT[:, h, :], AF.Square, scale=sc_sq
                                )
                        if DEBUG:
                            nc.sync.dma_start(
                                dbg["qtr"].ap().rearrange("(hm p) t -> p hm t", p=128),
                                QTr[:])
                            nc.sync.dma_start(
                                dbg["ktr"].ap().rearrange("(hm p) t -> p hm t", p=128),
                                KTr[:])

                # ===== Phase 3: attention (per batch, head, s-half) =====
                with tc.tile_pool(name="et", bufs=2) as et_pool, \
                     tc.tile_pool(name="sm", bufs=2) as sm_pool, \
                     tc.tile_pool(name="ps_sc", bufs=2, space="PSUM") as ps_sc, \
                     tc.tile_pool(name="ps_pv", bufs=2, space="PSUM") as ps_pv, \
                     tc.tile_pool(name="ps_cs", bufs=1, space="PSUM") as ps_cs, \
                     tc.tile_pool(name="ps_rb", bufs=1, space="PSUM") as ps_rb:
                    for b in range(B):
                        for h in range(NHL):
                            for sc in range(2):
                                s0 = b * S + sc * 512
                                ET = et_pool.tile([128, 8, 512], BF, tag="et")
                                for tt in range(8):
                                    t0 = b * S + tt * 128
                                    ps_s = ps_sc.tile([128, 512], F32, tag="ps_s")
                                    nc.tensor.matmul(
                                        ps_s[:], KTr[:, h, t0:t0 + 128],
                                        QTr[:, h, s0:s0 + 512],
                                        start=True, stop=False,
                                    )
                                    nc.tensor.matmul(
                                        ps_s[:], Ksq[:, h, t0:t0 + 128],
                                        Qsq[:, h, s0:s0 + 512],
                                        start=False, stop=True,
                                    )
                                    nc.scalar.activation(ET[:, tt, :], ps_s[:], AF.Exp)
                                ps_c = ps_cs.tile([1, 512], F32, tag="ps_c")
                                for tt in range(8):
                                    nc.tensor.matmul(
                                        ps_c[:], ones_c[:, 0:1], ET[:, tt, :],
                                        start=(tt == 0), stop=(tt == 7),
                                    )
                                rc = sm_pool.tile([1, 512], F32, tag="rc")
                                nc.vector.reciprocal(rc[:], ps_c[:])
                                ps_b = ps_rb.tile([128, 512], F32, tag="ps_b")
                                nc.tensor.matmul(
                                    ps_b[:], ones_r[:, :], rc[:], start=True, stop=True
                                )
                                rb = sm_pool.tile([128, 512], BF, tag="rb")
                                nc.scalar.copy(rb[:], ps_b[:])
                                ps_o = ps_pv.tile([128, 512], F32, tag="ps_o")
                                for tt in range(8):
                                    nc.tensor.matmul(
                                        ps_o[:],
                                        V[:, b * 8 + tt, h * 128:(h + 1) * 128],
                                        ET[:, tt, :],
                                        start=(tt == 0), stop=(tt == 7),
                                    )
                                t1 = sm_pool.tile([128, 512], F32, tag="t1")
                                t2 = sm_pool.tile([128, 512], F32, tag="t2")
                                nc.vector.tensor_mul(t1[:], ps_o[:], rb[:])
                                nc.vector.tensor_mul(t2[:], t1[:], t1[:])
                                nc.vector.scalar_tensor_tensor(
                                    OT[:, h, s0:s0 + 512], t2[:], HAD, t1[:],
                                    ALU.mult, ALU.add,
                                )
                if DEBUG:
                    nc.sync.dma_start(
                        dbg["ot"].ap().rearrange("(hm p) t -> p hm t", p=128), OT[:]
                    )

                # ===== Phase 4: wo partial + chunked ReduceScatter =====
                with tc.tile_pool(name="wo_w", bufs=1) as wo_w, \
                     tc.tile_pool(name="wo_ev", bufs=3) as wo_ev, \
                     tc.tile_pool(name="ps_wo", bufs=3, space="PSUM") as ps_wo:
                    WOT = wo_w.tile([128, NHL, D], BF, tag="wot")
                    nc.sync.dma_start(
                        WOT[:], woT_e.ap().rearrange("(hm p) n -> p hm n", p=128))
                    for k in range(NK):
                        bounce = dram.tile([CHUNK, D], BF, tag=f"attn_b{k}")
                        for ttl in range(4):
                            tok0 = k * CHUNK + ttl * 128
                            ao = wo_ev.tile([128, D], BF, tag="ao")
                            for nn in range(4):
                                ps_w = ps_wo.tile([128, 512], F32, tag="ps_w")
                                for hm in range(NHL):
                                    nc.tensor.matmul(
                                        ps_w[:],
                                        OT[:, hm, tok0:tok0 + 128],
                                        WOT[:, hm, nn * 512:(nn + 1) * 512],
                                        start=(hm == 0), stop=(hm == NHL - 1),
                                    )
                                nc.vector.tensor_copy(
                                    ao[:, nn * 512:(nn + 1) * 512], ps_w[:])
                            nc.sync.dma_start(
                                bounce[ttl * 128:(ttl + 1) * 128, :], ao[:])
                        rs_out = dram.tile([SH, D], BF, tag=f"attn_rs{k}")
                        nc.gpsimd.collective_compute(
                            "ReduceScatter", ALU.add, replica_groups=RG,
                            ins=[bounce.opt()], outs=[rs_out.opt()],
                        )
                        attn_rs.append(rs_out)
                        if DEBUG:
                            nc.sync.dma_start(
                                dbg["attn"][k * CHUNK:(k + 1) * CHUNK, :], bounce[:])

            # ===== Phases 5-7: residual + norm2 + AG + FFN + RS + output =====
            with tc.tile_pool(name="hres", bufs=1) as hres, \
                 tc.tile_pool(name="n2a", bufs=2) as n2a, \
                 tc.tile_pool(name="n2b", bufs=1) as n2b, \
                 tc.tile_pool(name="n2t", bufs=2) as n2t, \
                 tc.tile_pool(name="ffn_w13", bufs=3) as ffn_w13, \
                 tc.tile_pool(name="ffn_w2", bufs=1) as ffn_w2, \
                 tc.tile_pool(name="xn2a", bufs=1) as xn2a_pool, \
                 tc.tile_pool(name="ffn_act", bufs=2) as ffn_act, \
                 tc.tile_pool(name="ffn_ev", bufs=2) as ffn_ev, \
                 tc.tile_pool(name="ps_tr", bufs=2, space="PSUM") as ps_tr, \
                 tc.tile_pool(name="ps_g", bufs=2, space="PSUM") as ps_g, \
                 tc.tile_pool(name="ps_u", bufs=2, space="PSUM") as ps_u, \
                 tc.tile_pool(name="ps_y", bufs=2, space="PSUM") as ps_y:
                W2 = ffn_w2.tile([128, NDF, D], BF, tag="w2")
                nc.sync.dma_start(
                    W2[:], w2T_e.ap().rearrange("(df p) n -> p df n", p=128))

                for k in range(NK):
                    Hk = hres.tile([64, D], F32, tag=f"h{k}", name=f"h{k}")
                    ha = n2a.tile([64, D], BF, tag="ha")
                    xsk = n2a.tile([64, D], BF, tag="xsk")
                    nc.sync.dma_start(ha[:], attn_rs[k][:])
                    nc.sync.dma_start(xsk[:], xs_e[k * SH:(k + 1) * SH, :])
                    nc.vector.tensor_add(Hk[:], xsk[:], ha[:])
                    if DEBUG:
                        nc.sync.dma_start(dbg["h"][k * SH:(k + 1) * SH, :], Hk[:])
                    # RMSNorm2 on the 64-token shard
                    scr = n2b.tile([64, D], BF, tag="scr")
                    ssq2 = n2b.tile([64, 1], F32, tag="ssq2")
                    nc.scalar.activation(scr[:], Hk[:], AF.Square, accum_out=ssq2[:])
                    sr2 = n2b.tile([64, 1], F32, tag="sr2")
                    nc.scalar.activation(
                        sr2[:], ssq2[:], AF.Sqrt, scale=1.0 / D, bias=eps_t[0:64, :])
                    r2 = n2b.tile([64, 1], F32, tag="r2")
                    nc.vector.reciprocal(r2[:], sr2[:])
                    xn2 = n2a.tile([64, D], BF, tag="xn2")
                    nc.vector.tensor_scalar_mul(xn2[:], Hk[:], r2[:])
                    # transpose to [D, 64] (pack 8 transposes per PSUM bank)
                    xn2t = n2t.tile([128, DT_TILES, 64], BF, tag="xn2t")
                    for half in range(2):
                        ps_t = ps_tr.tile([128, 1024], BF, tag="ps_t")
                        for j in range(8):
                            dch = half * 8 + j
                            nc.tensor.matmul(
                                ps_t[:, j * 64:(j + 1) * 64],
                                xn2[:, dch * 128:(dch + 1) * 128],
                                ident[0:64, 0:64],
                                is_transpose=True,
                                start=(j == 0), stop=(j == 7),
                            )
                        for j in range(8):
                            nc.scalar.copy(
                                xn2t[:, half * 8 + j, :], ps_t[:, j * 64:(j + 1) * 64])
                    ag_in = dram.tile([D, SH], BF, tag=f"ag_in{k}")
                    nc.sync.dma_start(
                        ag_in.rearrange("(dt p) j -> p dt j", p=128), xn2t[:])
                    ag_out = dram.tile(
                        [N_CORES * D, SH], BF, tag=f"ag_out{k}", addr_space="Shared")
                    nc.gpsimd.collective_compute(
                        "AllGather", ALU.bypass, replica_groups=RG,
                        ins=[ag_in.opt()], outs=[ag_out.opt()],
                    )

                    # ===== Phase 6: FFN on token chunk k (512 tokens) =====
                    XN2A = xn2a_pool.tile([128, DT_TILES, N_CORES, SH], BF, tag="xn2a")
                    ag_view = ag_out.rearrange(
                        "(g dt p) j -> g p dt j", p=128, dt=DT_TILES)
                    for g in range(N_CORES):
                        nc.sync.dma_start(XN2A[:, :, g, :], ag_view[g])
                    ACT_K = ffn_act.tile([128, NDF, CHUNK], BF, tag="actk")
                    for df in range(NDF):
                        W1df = ffn_w13.tile([128, DT_TILES, 128], BF, tag="w1df")
                        W3df = ffn_w13.tile([128, DT_TILES, 128], BF, tag="w3df")
                        nc.sync.dma_start(
                            W1df[:],
                            w1T_e[df].rearrange("(dt p) m -> p dt m", p=128))
                        nc.sync.dma_start(
                            W3df[:],
                            w3T_e[df].rearrange("(dt p) m -> p dt m", p=128))
                        psg = ps_g.tile([128, 512], F32, tag="psg")
                        psu = ps_u.tile([128, 512], F32, tag="psu")
                        for dt in range(DT_TILES):
                            rhs = XN2A[:, dt, :, :]
                            nc.tensor.matmul(
                                psg[:], W1df[:, dt, :], rhs,
                                start=(dt == 0), stop=(dt == DT_TILES - 1),
                            )
                            nc.tensor.matmul(
                                psu[:], W3df[:, dt, :], rhs,
                                start=(dt == 0), stop=(dt == DT_TILES - 1),
                            )
                        sg = ffn_ev.tile([128, 512], BF, tag="sg")
                        nc.scalar.activation(sg[:], psg[:], AF.Silu)
                        nc.vector.tensor_mul(ACT_K[:, df, :], psu[:], sg[:])
                    ffn_bounce = dram.tile([CHUNK, D], BF, tag=f"ffn_b{k}")
                    for ttl in range(4):
                        yo = ffn_ev.tile([128, D], BF, tag="yo")
                        for nn in range(4):
                            psy = ps_y.tile([128, 512], F32, tag="psy")
                            for df in range(NDF):
                                nc.tensor.matmul(
                                    psy[:],
                                    ACT_K[:, df, ttl * 128:(ttl + 1) * 128],
                                    W2[:, df, nn * 512:(nn + 1) * 512],
                                    start=(df == 0), stop=(df == NDF - 1),
                                )
                            nc.scalar.copy(yo[:, nn * 512:(nn + 1) * 512], psy[:])
                        nc.sync.dma_start(
                            ffn_bounce[ttl * 128:(ttl + 1) * 128, :], yo[:])
                    ffn_rs = dram.tile([SH, D], BF, tag=f"ffn_rs{k}")
                    nc.gpsimd.collective_compute(
                        "ReduceScatter", ALU.add, replica_groups=RG,
                        ins=[ffn_bounce.opt()], outs=[ffn_rs.opt()],
                    )
                    if DEBUG:
                        nc.sync.dma_start(
                            dbg["y"][k * CHUNK:(k + 1) * CHUNK, :], ffn_bounce[:])
                    # ===== Phase 7: final residual + output =====
                    yk = n2a.tile([64, D], BF, tag="yk")
                    nc.sync.dma_start(yk[:], ffn_rs[:])
                    ok = n2b.tile([64, D], F32, tag="ok")
                    nc.vector.tensor_add(ok[:], Hk[:], yk[:])
                    nc.sync.dma_start(out_e[k * SH:(k + 1) * SH, :], ok[:])

    nc.compile()
    return nc


_NC_CACHE = None


def _get_nc():
    global _NC_CACHE
    if _NC_CACHE is None:
        _NC_CACHE = build_nc()
    return _NC_CACHE


def prep_inputs(x, norm1_w, norm2_w, wq, wk, wv, wo, gate_w, w1, w3, w2):
    """Build the 8 per-core input maps (host-side sharding + layout prep)."""
    x2d = np.ascontiguousarray(np.asarray(x, np.float32).reshape(T, D))
    xT = np.ascontiguousarray(x2d.T).astype(BF16)
    pi = np.concatenate([np.arange(0, HD, 2), np.arange(1, HD, 2)])
    inv = 1.0 / (10000.0 ** (np.arange(0, HD, 2, dtype=np.float64) / HD))
    ang = np.arange(S, dtype=np.float64)[:, None] * inv[None, :]   # [S, 64]
    Ct = np.tile(np.cos(ang).T, (1, B)).astype(np.float32)          # [64, T]
    St = np.tile(np.sin(ang).T, (1, B)).astype(np.float32)
    qs = 1.0 / np.sqrt(HD)
    rCq = (Ct * qs).astype(BF16)
    rSq = (St * qs).astype(BF16)
    rCk = Ct.astype(BF16)
    rSk = St.astype(BF16)

    n1 = np.asarray(norm1_w, np.float32)
    n2 = np.asarray(norm2_w, np.float32)
    wq = np.asarray(wq, np.float32)
    wk = np.asarray(wk, np.float32)
    wv = np.asarray(wv, np.float32)
    wo = np.asarray(wo, np.float32)
    w1 = np.asarray(w1, np.float32)
    w3 = np.asarray(w3, np.float32)
    w2 = np.asarray(w2, np.float32)

    in_maps = []
    for c in range(N_CORES):
        r0, r1 = c * DQ, (c + 1) * DQ
        wq_c = (wq[r0:r1, :] * n1[None, :]).reshape(NHL, HD, D)[:, pi, :].reshape(DQ, D)
        wk_c = (wk[r0:r1, :] * n1[None, :]).reshape(NHL, HD, D)[:, pi, :].reshape(DQ, D)
        wv_c = wv[r0:r1, :] * n1[None, :]
        f0, f1 = c * DFL, (c + 1) * DFL
        xs_c = np.concatenate(
            [x2d[k * CHUNK + c * SH: k * CHUNK + (c + 1) * SH] for k in range(NK)],
            axis=0,
        )
        w1_c = (w1[f0:f1, :] * n2[None, :]).T.astype(BF16)   # [D, DFL]
        w3_c = (w3[f0:f1, :] * n2[None, :]).T.astype(BF16)
        # df-major: [NDF, D, 128]
        w1_df = np.ascontiguousarray(w1_c.reshape(D, NDF, 128).transpose(1, 0, 2))
        w3_df = np.ascontiguousarray(w3_c.reshape(D, NDF, 128).transpose(1, 0, 2))
        in_maps.append({
            "xT": xT,
            "xs": xs_c.astype(BF16),
            "wqT": np.ascontiguousarray(wq_c.T).astype(BF16),
            "wkT": np.ascontiguousarray(wk_c.T).astype(BF16),
            "wvT": np.ascontiguousarray(wv_c.T).astype(BF16),
            "woT": np.ascontiguousarray(wo[:, r0:r1].T).astype(BF16),
            "w1T": w1_df,
            "w3T": w3_df,
            "w2T": np.ascontiguousarray(w2[:, f0:f1].T).astype(BF16),
            "ropeCq": rCq, "ropeSq": rSq, "ropeCk": rCk, "ropeSk": rSk,
        })
    return in_maps


def unshard_output(results):
    out = np.empty((T, D), np.float32)
    for c in range(N_CORES):
        oc = results[c]["out"]
        for k in range(NK):
            out[k * CHUNK + c * SH: k * CHUNK + (c + 1) * SH] = oc[k * SH:(k + 1) * SH]
    return out.reshape(B, S, D)


def run(in_maps, trace=False):
    nc = _get_nc()
    return run_bass_kernel_spmd(nc, in_maps, core_ids=list(range(N_CORES)), trace=trace)


def kernel(**inputs):
    in_maps = prep_inputs(**inputs)
    res = run(in_maps, trace=False)
    return unshard_output(res.results)


# revision 12
# speedup vs baseline: 1.1194x; 1.1194x over previous
"""Trainium2 Bass kernel for nn_AGITransformer140B (8-core tensor-parallel).

Transformer block: h = x + Attn(RMSNorm(x)); out = h + SwiGLU(RMSNorm(h)).

Key simplification: the reference's second attention pass uses
rotate_half(Q), rotate_half(K) — which preserves both Q·K and Q²·K² inner
products exactly, so out2 == out1 and the sigmoid gate is a no-op.  Only one
attention pass is computed.

Sharding: TP-8 over heads (2/core) and d_ff (1024/core).  Partial attention
and FFN outputs are ReduceScattered over tokens (4 chunks of 512 tokens,
each core owning 64-token slices); RMSNorm2 + residuals run on the local
token shard; normed activations are AllGathered (feature-major) for the FFN.

Layouts (per core):
  xT       [D=2048, T=2048]  bf16 feature-major input (host-transposed)
  xs       [256, 2048]       bf16 x token-shard, local (k,j) order
  wqT/wkT  [2048, 256]  bf16 (norm1_w folded, head-dim pi-permuted: evens|odds)
  wvT      [2048, 256]  bf16 (norm1_w folded)
  woT      [256, 2048]  bf16
  w1T/w3T  [8, 2048, 128] bf16 df-major (norm2_w folded)
  w2T      [1024, 2048] bf16
  ropeC*/S* [64, 2048]  bf16 rope tables (Q tables pre-scaled by 1/sqrt(hd))
Output: out [256, 2048] f32, core c owns tokens {k*512 + c*64 + j}.
"""

import os
import sys
import types

sys.path.insert(0, "/opt/trn_rl_repo")

# ---- NTFF profile hook (boot() skips it: antenv stub lacks axon_hooks) ----
if "antenv.axon_hooks" not in sys.modules:
    _hooks_mod = types.ModuleType("antenv.axon_hooks")
    _HOOK = [None]
    _hooks_mod.set_axon_ntff_profile_hook = lambda h: _HOOK.__setitem__(0, h)
    _hooks_mod.get_axon_ntff_profile_hook = lambda: _HOOK[0]
    sys.modules["antenv.axon_hooks"] = _hooks_mod
    try:
        from trn_agent_boot.trn_boot import _ntff_profile_via_ctypes

        _hooks_mod.set_axon_ntff_profile_hook(
            _ntff_profile_via_ctypes("/opt/axon/libaxon_pjrt.so")
        )
    except Exception:
        pass

import ml_dtypes
import numpy as np

import concourse.bass as bass
import concourse.mybir as mybir
import concourse.tile as tile
from concourse import bacc
from concourse.bass_utils import run_bass_kernel_spmd
from concourse.masks import make_identity

BF16 = ml_dtypes.bfloat16
F32 = mybir.dt.float32
BF = mybir.dt.bfloat16
AF = mybir.ActivationFunctionType
ALU = mybir.AluOpType

N_CORES = 8
B, S, D, NH, HD, DFF = 2, 1024, 2048, 16, 128, 8192
T = B * S                      # 2048 tokens
NHL = NH // N_CORES            # 2 heads per core
DQ = NHL * HD                  # 256
DFL = DFF // N_CORES           # 1024
NDF = DFL // 128               # 8 dff tiles per core
NK = 4                         # token chunks for RS pipelining
CHUNK = T // NK                # 512
SH = CHUNK // N_CORES          # 64 tokens per (chunk, core)
DT_TILES = D // 128            # 16
EPS = 1e-6
LAM = 0.1
HAD = 0.05
SQ_SCALE = float(np.sqrt(LAM * np.sqrt(HD)))   # fold lam*sqrt(hd) into Q^2
DEBUG = bool(int(os.environ.get("KERNEL_DEBUG", "0")))


def build_nc():
    nc = bacc.Bacc("TRN2", target_bir_lowering=False, debug=False)

    xT_e = nc.declare_dram_parameter("xT", [D, T], BF, isOutput=False)
    xs_e = nc.declare_dram_parameter("xs", [NK * SH, D], BF, isOutput=False)
    wqT_e = nc.declare_dram_parameter("wqT", [D, DQ], BF, isOutput=False)
    wkT_e = nc.declare_dram_parameter("wkT", [D, DQ], BF, isOutput=False)
    wvT_e = nc.declare_dram_parameter("wvT", [D, DQ], BF, isOutput=False)
    woT_e = nc.declare_dram_parameter("woT", [DQ, D], BF, isOutput=False)
    w1T_e = nc.declare_dram_parameter("w1T", [NDF, D, 128], BF, isOutput=False)
    w3T_e = nc.declare_dram_parameter("w3T", [NDF, D, 128], BF, isOutput=False)
    w2T_e = nc.declare_dram_parameter("w2T", [DFL, D], BF, isOutput=False)
    rCq_e = nc.declare_dram_parameter("ropeCq", [64, T], BF, isOutput=False)
    rSq_e = nc.declare_dram_parameter("ropeSq", [64, T], BF, isOutput=False)
    rCk_e = nc.declare_dram_parameter("ropeCk", [64, T], BF, isOutput=False)
    rSk_e = nc.declare_dram_parameter("ropeSk", [64, T], BF, isOutput=False)
    out_e = nc.declare_dram_parameter("out", [NK * SH, D], F32, isOutput=True)

    dbg = {}
    if DEBUG:
        dbg["xnt"] = nc.declare_dram_parameter("dbg_xnt", [D, T], BF, isOutput=True)
        dbg["qtr"] = nc.declare_dram_parameter("dbg_qtr", [DQ, T], BF, isOutput=True)
        dbg["ktr"] = nc.declare_dram_parameter("dbg_ktr", [DQ, T], BF, isOutput=True)
        dbg["v"] = nc.declare_dram_parameter("dbg_v", [T, DQ], BF, isOutput=True)
        dbg["ot"] = nc.declare_dram_parameter("dbg_ot", [DQ, T], BF, isOutput=True)
        dbg["attn"] = nc.declare_dram_parameter("dbg_attn", [T, D], BF, isOutput=True)
        dbg["h"] = nc.declare_dram_parameter("dbg_h", [NK * SH, D], F32, isOutput=True)
        dbg["y"] = nc.declare_dram_parameter("dbg_y", [T, D], BF, isOutput=True)

    RG = [list(range(N_CORES))]

    with tile.TileContext(nc) as tc:
        with tc.tile_pool(name="const", bufs=1) as const, \
             tc.tile_pool(name="dram", bufs=1, space="DRAM") as dram:
            ident = const.tile([128, 128], BF)
            make_identity(nc, ident)
            ones_c = const.tile([128, 1], BF)     # colsum lhsT (K=128, M=1)
            nc.vector.memset(ones_c[:], 1.0)
            ones_r = const.tile([1, 128], F32)    # bcast lhsT (K=1, M=128)
            nc.vector.memset(ones_r[:], 1.0)
            eps_t = const.tile([128, 1], F32)
            nc.vector.memset(eps_t[:], EPS)

            rCq = const.tile([64, T], BF)
            rSq = const.tile([64, T], BF)
            rCk = const.tile([64, T], BF)
            rSk = const.tile([64, T], BF)
            nc.sync.dma_start(rCq[:], rCq_e[:])
            nc.sync.dma_start(rSq[:], rSq_e[:])
            nc.sync.dma_start(rCk[:], rCk_e[:])
            nc.sync.dma_start(rSk[:], rSk_e[:])

            attn_rs = []
            with tc.tile_pool(name="aops", bufs=1) as aops:
                V = aops.tile([128, 16, DQ], BF, tag="v")
                QTr = aops.tile([128, NHL, T], BF, tag="qtr")
                KTr = aops.tile([128, NHL, T], BF, tag="ktr")
                Qsq = aops.tile([128, NHL, T], BF, tag="qsq")
                Ksq = aops.tile([128, NHL, T], BF, tag="ksq")
                OT = aops.tile([128, NHL, T], BF, tag="ot")

                with tc.tile_pool(name="xnt_pool", bufs=1) as xnt_pool:
                    XNT = xnt_pool.tile([128, DT_TILES, T], BF, tag="xnt")
                    xT_view = xT_e.ap().rearrange("(dt p) t -> dt p t", p=128)
                    for dt in range(DT_TILES):
                        nc.sync.dma_start(XNT[:, dt, :], xT_view[dt])
                    RB1 = xnt_pool.tile([128, T], BF, tag="rb1")
                    RSTM = xnt_pool.tile([128, 16], F32, tag="rstm")
                    # ===== Phase 1: RMSNorm1 (feature-major, in-place) =====
                    with tc.tile_pool(name="n1", bufs=3) as n1, \
                         tc.tile_pool(name="n1s", bufs=1) as n1s, \
                         tc.tile_pool(name="ps_n1", bufs=1, space="PSUM") as ps_n1, \
                         tc.tile_pool(name="ps_rb1", bufs=2, space="PSUM") as ps_rb1:
                        ssq_ps = [
                            ps_n1.tile([1, 512], F32, tag=f"ssq{nn}", name=f"ssq{nn}")
                            for nn in range(4)
                        ]
                        for dt in range(DT_TILES):
                            sq = n1.tile([128, T], BF, tag="sq")
                            nc.scalar.activation(sq[:], XNT[:, dt, :], AF.Square)
                            for nn in range(4):
                                nc.tensor.matmul(
                                    ssq_ps[nn][:], ones_c[:, 0:1],
                                    sq[:, nn * 512:(nn + 1) * 512],
                                    start=(dt == 0), stop=(dt == DT_TILES - 1),
                                )
                        rstd = n1s.tile([1, T], F32, tag="rstd")
                        sr = n1s.tile([1, T], F32, tag="sr")
                        for nn in range(4):
                            nc.scalar.activation(
                                sr[:, nn * 512:(nn + 1) * 512], ssq_ps[nn][:],
                                AF.Sqrt, scale=1.0 / D, bias=eps_t[0:1, :],
                            )
                        nc.vector.reciprocal(rstd[:], sr[:])
                        for nn in range(4):
                            rb_ps = ps_rb1.tile([128, 512], F32, tag="rbps")
                            nc.tensor.matmul(
                                rb_ps[:], ones_r[:, :],
                                rstd[:, nn * 512:(nn + 1) * 512],
                                start=True, stop=True,
                            )
                            nc.scalar.copy(RB1[:, nn * 512:(nn + 1) * 512], rb_ps[:])
                        tm_ps = ps_rb1.tile([128, 16], F32, tag="tmps")
                        for tt in range(16):
                            nc.tensor.matmul(
                                tm_ps[:, tt:tt + 1],
                                rstd[0:1, tt * 128:(tt + 1) * 128],
                                ones_r[0:1, 0:1],
                                start=(tt == 0), stop=(tt == 15),
                            )
                        nc.scalar.copy(RSTM[:], tm_ps[:])

                    # ===== Phase 2: QKV projections + RoPE =====
                    with tc.tile_pool(name="qkv_w", bufs=1) as qkv_w, \
                         tc.tile_pool(name="qkv_raw", bufs=1) as qkv_raw, \
                         tc.tile_pool(name="rope_tmp", bufs=1) as rtmp, \
                         tc.tile_pool(name="ps_qkv", bufs=2, space="PSUM") as ps_qkv:
                        WQ = qkv_w.tile([128, DT_TILES, DQ], BF, tag="wq")
                        WK = qkv_w.tile([128, DT_TILES, DQ], BF, tag="wk")
                        WV = qkv_w.tile([128, DT_TILES, DQ], BF, tag="wv")
                        nc.sync.dma_start(
                            WQ[:], wqT_e.ap().rearrange("(dt p) m -> p dt m", p=128))
                        nc.sync.dma_start(
                            WK[:], wkT_e.ap().rearrange("(dt p) m -> p dt m", p=128))
                        nc.sync.dma_start(
                            WV[:], wvT_e.ap().rearrange("(dt p) m -> p dt m", p=128))

                        QTraw = qkv_raw.tile([128, NHL, T], BF, tag="qraw")
                        KTraw = qkv_raw.tile([128, NHL, T], BF, tag="kraw")
                        for (W, OUT) in ((WQ, QTraw), (WK, KTraw)):
                            for hm in range(NHL):
                                for nq in range(4):
                                    ps = ps_qkv.tile([128, 512], F32, tag="qk_ps")
                                    for dt in range(DT_TILES):
                                        nc.tensor.matmul(
                                            ps[:],
                                            W[:, dt, hm * 128:(hm + 1) * 128],
                                            XNT[:, dt, nq * 512:(nq + 1) * 512],
                                            start=(dt == 0), stop=(dt == DT_TILES - 1),
                                        )
                                    nc.vector.tensor_mul(
                                        OUT[:, hm, nq * 512:(nq + 1) * 512], ps[:],
                                        RB1[:, nq * 512:(nq + 1) * 512])
                        for tt in range(16):
                            ps = ps_qkv.tile([128, DQ], F32, tag="v_ps")
                            for dt in range(DT_TILES):
                                nc.tensor.matmul(
                                    ps[:],
                                    XNT[:, dt, tt * 128:(tt + 1) * 128],
                                    WV[:, dt, :],
                                    start=(dt == 0), stop=(dt == DT_TILES - 1),
                                )
                            nc.vector.tensor_scalar_mul(
                                V[:, tt, :], ps[:], RSTM[:, tt:tt + 1])
                        if DEBUG:
                            nc.sync.dma_start(
                                dbg["v"].ap().rearrange("(tt p) m -> p tt m", p=128),
                                V[:],
                            )

                        # RoPE (pi layout: rows 0:64 = even dims, 64:128 = odds)
                        for (RAW, ROT, SQT, CC, SS, sc_sq) in (
                            (QTraw, QTr, Qsq, rCq, rSq, SQ_SCALE),
                            (KTraw, KTr, Ksq, rCk, rSk, 1.0),
                        ):
                            for h in range(NHL):
                                x1 = RAW[0:64, h, :]
                                x2c = rtmp.tile([64, T], BF, tag="x2c")
                                nc.vector.tensor_copy(x2c[:], RAW[64:128, h, :])
                                tA = rtmp.tile([64, T], BF, tag="ta")
                                tB = rtmp.tile([64, T], BF, tag="tb")
                                nc.vector.tensor_mul(tA[:], x1, CC[:])
                                nc.vector.tensor_mul(tB[:], x2c[:], SS[:])
                                nc.vector.tensor_sub(ROT[0:64, h, :], tA[:], tB[:])
                                tC = rtmp.tile([64, T], BF, tag="tc")
                                tD = rtmp.tile([64, T], BF, tag="td")
                                nc.vector.tensor_mul(tC[:], x1, SS[:])
                                nc.vector.tensor_mul(tD[:], x2c[:], CC[:])
                                hi = rtmp.tile([64, T], BF, tag="hi")
                                nc.vector.tensor_add(hi[:], tC[:], tD[:])
                                nc.vector.tensor_copy(ROT[64:128, h, :], hi[:])
                                nc.scalar.activation(
                                    SQT[:, h, :], ROT[:, h, :], AF.Square, scale=sc_sq
                                )
                        if DEBUG:
                            nc.sync.dma_start(
                                dbg["qtr"].ap().rearrange("(hm p) t -> p hm t", p=128),
                                QTr[:])
                            nc.sync.dma_start(
                                dbg["ktr"].ap().rearrange("(hm p) t -> p hm t", p=128),
                                KTr[:])

                # ===== Phase 3+4: attention interleaved with wo + RS =====
                with tc.tile_pool(name="wo_w", bufs=1) as wo_w, \
                     tc.tile_pool(name="wo_ev", bufs=3) as wo_ev, \
                     tc.tile_pool(name="ps_wo", bufs=2, space="PSUM") as ps_wo:
                    WOT = wo_w.tile([128, NHL, D], BF, tag="wot")
                    nc.sync.dma_start(
                        WOT[:], woT_e.ap().rearrange("(hm p) n -> p hm n", p=128))
                    with tc.tile_pool(name="et", bufs=2) as et_pool, \
                         tc.tile_pool(name="sm", bufs=2) as sm_pool, \
                         tc.tile_pool(name="ps_sc", bufs=2, space="PSUM") as ps_sc, \
                         tc.tile_pool(name="ps_pv", bufs=2, space="PSUM") as ps_pv, \
                         tc.tile_pool(name="ps_cs", bufs=1, space="PSUM") as ps_cs, \
                         tc.tile_pool(name="ps_rb", bufs=1, space="PSUM") as ps_rb:
                        for b in range(B):
                            for h in range(NHL):
                                for sc in range(2):
                                    s0 = b * S + sc * 512
                                    ET = et_pool.tile([128, 8, 512], BF, tag="et")
                                    for tt in range(8):
                                        t0 = b * S + tt * 128
                                        ps_s = ps_sc.tile([128, 512], F32, tag="ps_s")
                                        nc.tensor.matmul(
                                            ps_s[:], KTr[:, h, t0:t0 + 128],
                                            QTr[:, h, s0:s0 + 512],
                                            start=True, stop=False,
                                        )
                                        nc.tensor.matmul(
                                            ps_s[:], Ksq[:, h, t0:t0 + 128],
                                            Qsq[:, h, s0:s0 + 512],
                                            start=False, stop=True,
                                        )
                                        nc.scalar.activation(
                                            ET[:, tt, :], ps_s[:], AF.Exp)
                                    ps_c = ps_cs.tile([1, 512], F32, tag="ps_c")
                                    for tt in range(8):
                                        nc.tensor.matmul(
                                            ps_c[:], ones_c[:, 0:1], ET[:, tt, :],
                                            start=(tt == 0), stop=(tt == 7),
                                        )
                                    rc = sm_pool.tile([1, 512], F32, tag="rc")
                                    nc.vector.reciprocal(rc[:], ps_c[:])
                                    ps_o = ps_pv.tile([128, 512], F32, tag="ps_o")
                                    for tt in range(8):
                                        nc.tensor.matmul(
                                            ps_o[:],
                                            V[:, b * 8 + tt, h * 128:(h + 1) * 128],
                                            ET[:, tt, :],
                                            start=(tt == 0), stop=(tt == 7),
                                        )
                                    ps_b = ps_rb.tile([128, 512], F32, tag="ps_b")
                                    nc.tensor.matmul(
                                        ps_b[:], ones_r[:, :], rc[:],
                                        start=True, stop=True,
                                    )
                                    rb = sm_pool.tile([128, 512], BF, tag="rb")
                                    nc.scalar.copy(rb[:], ps_b[:])
                                    t1 = sm_pool.tile([128, 512], F32, tag="t1")
                                    t2 = sm_pool.tile([128, 512], F32, tag="t2")
                                    nc.vector.tensor_mul(t1[:], ps_o[:], rb[:])
                                    nc.vector.tensor_mul(t2[:], t1[:], t1[:])
                                    nc.vector.scalar_tensor_tensor(
                                        OT[:, h, s0:s0 + 512], t2[:], HAD, t1[:],
                                        ALU.mult, ALU.add,
                                    )
                            # wo partial + RS for this batch's two token chunks
                            for k in (2 * b, 2 * b + 1):
                                bounce = dram.tile([CHUNK, D], BF, tag=f"attn_b{k}")
                                for ttl in range(4):
                                    tok0 = k * CHUNK + ttl * 128
                                    ao = wo_ev.tile([128, D], BF, tag="ao")
                                    for nn in range(4):
                                        ps_w = ps_wo.tile([128, 512], F32, tag="ps_w")
                                        for hm in range(NHL):
                                            nc.tensor.matmul(
                                                ps_w[:],
                                                OT[:, hm, tok0:tok0 + 128],
                                                WOT[:, hm, nn * 512:(nn + 1) * 512],
                                                start=(hm == 0), stop=(hm == NHL - 1),
                                            )
                                        nc.vector.tensor_copy(
                                            ao[:, nn * 512:(nn + 1) * 512], ps_w[:])
                                    nc.sync.dma_start(
                                        bounce[ttl * 128:(ttl + 1) * 128, :], ao[:])
                                rs_out = dram.tile([SH, D], BF, tag=f"attn_rs{k}")
                                nc.gpsimd.collective_compute(
                                    "ReduceScatter", ALU.add, replica_groups=RG,
                                    ins=[bounce.opt()], outs=[rs_out.opt()],
                                )
                                attn_rs.append(rs_out)
                                if DEBUG:
                                    nc.sync.dma_start(
                                        dbg["attn"][k * CHUNK:(k + 1) * CHUNK, :],
                                        bounce[:])
                    if DEBUG:
                        nc.sync.dma_start(
                            dbg["ot"].ap().rearrange("(hm p) t -> p hm t", p=128),
                            OT[:])

            # ===== Phases 5-7: residual + norm2 + AG + FFN + RS + output =====
            with tc.tile_pool(name="hres", bufs=1) as hres, \
                 tc.tile_pool(name="n2a", bufs=2) as n2a, \
                 tc.tile_pool(name="n2b", bufs=1) as n2b, \
                 tc.tile_pool(name="n2t", bufs=2) as n2t, \
                 tc.tile_pool(name="ffn_w13", bufs=3) as ffn_w13, \
                 tc.tile_pool(name="ffn_w2", bufs=1) as ffn_w2, \
                 tc.tile_pool(name="xn2a", bufs=1) as xn2a_pool, \
                 tc.tile_pool(name="ffn_act", bufs=2) as ffn_act, \
                 tc.tile_pool(name="ffn_ev", bufs=2) as ffn_ev, \
                 tc.tile_pool(name="ps_tr", bufs=2, space="PSUM") as ps_tr, \
                 tc.tile_pool(name="ps_g", bufs=2, space="PSUM") as ps_g, \
                 tc.tile_pool(name="ps_u", bufs=2, space="PSUM") as ps_u, \
                 tc.tile_pool(name="ps_y", bufs=2, space="PSUM") as ps_y:
                W2 = ffn_w2.tile([128, NDF, D], BF, tag="w2")
                nc.sync.dma_start(
                    W2[:], w2T_e.ap().rearrange("(df p) n -> p df n", p=128))

                for k in range(NK):
                    Hk = hres.tile([64, D], F32, tag=f"h{k}", name=f"h{k}")
                    ha = n2a.tile([64, D], BF, tag="ha")
                    xsk = n2a.tile([64, D], BF, tag="xsk")
                    nc.sync.dma_start(ha[:], attn_rs[k][:])
                    nc.sync.dma_start(xsk[:], xs_e[k * SH:(k + 1) * SH, :])
                    nc.vector.tensor_add(Hk[:], xsk[:], ha[:])
                    if DEBUG:
                        nc.sync.dma_start(dbg["h"][k * SH:(k + 1) * SH, :], Hk[:])
                    # RMSNorm2 on the 64-token shard
                    scr = n2b.tile([64, D], BF, tag="scr")
                    ssq2 = n2b.tile([64, 1], F32, tag="ssq2")
                    nc.scalar.activation(scr[:], Hk[:], AF.Square, accum_out=ssq2[:])
                    sr2 = n2b.tile([64, 1], F32, tag="sr2")
                    nc.scalar.activation(
                        sr2[:], ssq2[:], AF.Sqrt, scale=1.0 / D, bias=eps_t[0:64, :])
                    r2 = n2b.tile([64, 1], F32, tag="r2")
                    nc.vector.reciprocal(r2[:], sr2[:])
                    xn2 = n2a.tile([64, D], BF, tag="xn2")
                    nc.vector.tensor_scalar_mul(xn2[:], Hk[:], r2[:])
                    # transpose to [D, 64] (pack 8 transposes per PSUM bank)
                    xn2t = n2t.tile([128, DT_TILES, 64], BF, tag="xn2t")
                    for half in range(2):
                        ps_t = ps_tr.tile([128, 1024], BF, tag="ps_t")
                        for j in range(8):
                            dch = half * 8 + j
                            nc.tensor.matmul(
                                ps_t[:, j * 64:(j + 1) * 64],
                                xn2[:, dch * 128:(dch + 1) * 128],
                                ident[0:64, 0:64],
                                is_transpose=True,
                                start=(j == 0), stop=(j == 7),
                            )
                        for j in range(8):
                            nc.scalar.copy(
                                xn2t[:, half * 8 + j, :], ps_t[:, j * 64:(j + 1) * 64])
                    ag_in = dram.tile([D, SH], BF, tag=f"ag_in{k}")
                    nc.sync.dma_start(
                        ag_in.rearrange("(dt p) j -> p dt j", p=128), xn2t[:])
                    ag_out = dram.tile(
                        [N_CORES * D, SH], BF, tag=f"ag_out{k}", addr_space="Shared")
                    nc.gpsimd.collective_compute(
                        "AllGather", ALU.bypass, replica_groups=RG,
                        ins=[ag_in.opt()], outs=[ag_out.opt()],
                    )

                    # ===== Phase 6: FFN on token chunk k (512 tokens) =====
                    XN2A = xn2a_pool.tile([128, DT_TILES, N_CORES, SH], BF, tag="xn2a")
                    ag_view = ag_out.rearrange(
                        "(g dt p) j -> g p dt j", p=128, dt=DT_TILES)
                    for g in range(N_CORES):
                        nc.sync.dma_start(XN2A[:, :, g, :], ag_view[g])
                    ACT_K = ffn_act.tile([128, NDF, CHUNK], BF, tag="actk")
                    for df in range(NDF):
                        W1df = ffn_w13.tile([128, DT_TILES, 128], BF, tag="w1df")
                        W3df = ffn_w13.tile([128, DT_TILES, 128], BF, tag="w3df")
                        nc.sync.dma_start(
                            W1df[:],
                            w1T_e[df].rearrange("(dt p) m -> p dt m", p=128))
                        nc.sync.dma_start(
                            W3df[:],
                            w3T_e[df].rearrange("(dt p) m -> p dt m", p=128))
                        psg = ps_g.tile([128, 512], F32, tag="psg")
                        psu = ps_u.tile([128, 512], F32, tag="psu")
                        for dt in range(DT_TILES):
                            rhs = XN2A[:, dt, :, :]
                            nc.tensor.matmul(
                                psg[:], W1df[:, dt, :], rhs,
                                start=(dt == 0), stop=(dt == DT_TILES - 1),
                            )
                            nc.tensor.matmul(
                                psu[:], W3df[:, dt, :], rhs,
                                start=(dt == 0), stop=(dt == DT_TILES - 1),
                            )
                        sg = ffn_ev.tile([128, 512], BF, tag="sg")
                        nc.scalar.activation(sg[:], psg[:], AF.Silu)
                        nc.vector.tensor_mul(ACT_K[:, df, :], psu[:], sg[:])
                    ffn_bounce = dram.tile([CHUNK, D], BF, tag=f"ffn_b{k}")
                    for ttl in range(4):
                        yo = ffn_ev.tile([128, D], BF, tag="yo")
                        for nn in range(4):
                            psy = ps_y.tile([128, 512], F32, tag="psy")
                            for df in range(NDF):
                                nc.tensor.matmul(
                                    psy[:],
                                    ACT_K[:, df, ttl * 128:(ttl + 1) * 128],
                                    W2[:, df, nn * 512:(nn + 1) * 512],
                                    start=(df == 0), stop=(df == NDF - 1),
                                )
                            nc.scalar.copy(yo[:, nn * 512:(nn + 1) * 512], psy[:])
                        nc.sync.dma_start(
                            ffn_bounce[ttl * 128:(ttl + 1) * 128, :], yo[:])
                    ffn_rs = dram.tile([SH, D], BF, tag=f"ffn_rs{k}")
                    nc.gpsimd.collective_compute(
                        "ReduceScatter", ALU.add, replica_groups=RG,
                        ins=[ffn_bounce.opt()], outs=[ffn_rs.opt()],
                    )
                    if DEBUG:
                        nc.sync.dma_start(
                            dbg["y"][k * CHUNK:(k + 1) * CHUNK, :], ffn_bounce[:])
                    # ===== Phase 7: final residual + output =====
                    yk = n2a.tile([64, D], BF, tag="yk")
                    nc.sync.dma_start(yk[:], ffn_rs[:])
                    ok = n2b.tile([64, D], F32, tag="ok")
                    nc.vector.tensor_add(ok[:], Hk[:], yk[:])
                    nc.sync.dma_start(out_e[k * SH:(k + 1) * SH, :], ok[:])

    nc.compile()
    return nc


_NC_CACHE = None


def _get_nc():
    global _NC_CACHE
    if _NC_CACHE is None:
        _NC_CACHE = build_nc()
    return _NC_CACHE


def prep_inputs(x, norm1_w, norm2_w, wq, wk, wv, wo, gate_w, w1, w3, w2):
    """Build the 8 per-core input maps (host-side sharding + layout prep)."""
    x2d = np.ascontiguousarray(np.asarray(x, np.float32).reshape(T, D))
    xT = np.ascontiguousarray(x2d.T).astype(BF16)
    pi = np.concatenate([np.arange(0, HD, 2), np.arange(1, HD, 2)])
    inv = 1.0 / (10000.0 ** (np.arange(0, HD, 2, dtype=np.float64) / HD))
    ang = np.arange(S, dtype=np.float64)[:, None] * inv[None, :]   # [S, 64]
    Ct = np.tile(np.cos(ang).T, (1, B)).astype(np.float32)          # [64, T]
    St = np.tile(np.sin(ang).T, (1, B)).astype(np.float32)
    qs = 1.0 / np.sqrt(HD)
    rCq = (Ct * qs).astype(BF16)
    rSq = (St * qs).astype(BF16)
    rCk = Ct.astype(BF16)
    rSk = St.astype(BF16)

    n1 = np.asarray(norm1_w, np.float32)
    n2 = np.asarray(norm2_w, np.float32)
    wq = np.asarray(wq, np.float32)
    wk = np.asarray(wk, np.float32)
    wv = np.asarray(wv, np.float32)
    wo = np.asarray(wo, np.float32)
    w1 = np.asarray(w1, np.float32)
    w3 = np.asarray(w3, np.float32)
    w2 = np.asarray(w2, np.float32)

    in_maps = []
    for c in range(N_CORES):
        r0, r1 = c * DQ, (c + 1) * DQ
        wq_c = (wq[r0:r1, :] * n1[None, :]).reshape(NHL, HD, D)[:, pi, :].reshape(DQ, D)
        wk_c = (wk[r0:r1, :] * n1[None, :]).reshape(NHL, HD, D)[:, pi, :].reshape(DQ, D)
        wv_c = wv[r0:r1, :] * n1[None, :]
        f0, f1 = c * DFL, (c + 1) * DFL
        xs_c = np.concatenate(
            [x2d[k * CHUNK + c * SH: k * CHUNK + (c + 1) * SH] for k in range(NK)],
            axis=0,
        )
        w1_c = (w1[f0:f1, :] * n2[None, :]).T.astype(BF16)   # [D, DFL]
        w3_c = (w3[f0:f1, :] * n2[None, :]).T.astype(BF16)
        # df-major: [NDF, D, 128]
        w1_df = np.ascontiguousarray(w1_c.reshape(D, NDF, 128).transpose(1, 0, 2))
        w3_df = np.ascontiguousarray(w3_c.reshape(D, NDF, 128).transpose(1, 0, 2))
        in_maps.append({
            "xT": xT,
            "xs": xs_c.astype(BF16),
            "wqT": np.ascontiguousarray(wq_c.T).astype(BF16),
            "wkT": np.ascontiguousarray(wk_c.T).astype(BF16),
            "wvT": np.ascontiguousarray(wv_c.T).astype(BF16),
            "woT": np.ascontiguousarray(wo[:, r0:r1].T).astype(BF16),
            "w1T": w1_df,
            "w3T": w3_df,
            "w2T": np.ascontiguousarray(w2[:, f0:f1].T).astype(BF16),
            "ropeCq": rCq, "ropeSq": rSq, "ropeCk": rCk, "ropeSk": rSk,
        })
    return in_maps


def unshard_output(results):
    out = np.empty((T, D), np.float32)
    for c in range(N_CORES):
        oc = results[c]["out"]
        for k in range(NK):
            out[k * CHUNK + c * SH: k * CHUNK + (c + 1) * SH] = oc[k * SH:(k + 1) * SH]
    return out.reshape(B, S, D)


def run(in_maps, trace=False):
    nc = _get_nc()
    return run_bass_kernel_spmd(nc, in_maps, core_ids=list(range(N_CORES)), trace=trace)


def kernel(**inputs):
    in_maps = prep_inputs(**inputs)
    res = run(in_maps, trace=False)
    return unshard_output(res.results)
